# revision 13
# baseline (speedup 1.0000x reference)
"""GQA attention (B=2, S=2048, 16 q heads / 4 kv heads, head_dim=128) with RoPE
and causal softmax, tensor-parallel over heads x data-parallel over batch on
8 Trainium2 NeuronCores.

Core c (0..7): batch b = c//4, TP rank r = c%4.
Each core computes, for its batch and its 4 q heads / 1 kv head:
  QT/KT/VT projections (transposed layout, contraction on partitions),
  RoPE in a host-permuted head_dim layout (evens then odds), so the pair
    rotation is a partition half-swap done by two SBUF-to-SBUF DMAs plus
    elementwise muls with [cos;cos] / [-sin;sin] tables -- no matmul,
  causal softmax without max-subtraction (scores bounded; exp fp32->bf16),
    with the denominator accumulated on DVE (bf16 tile adds) and a single
    ones-matmul per (head, q-block) for the cross-partition sum,
  P@V in transposed layout with a 128-granular causal diagonal (items of
    width 512-128j per diagonal key tile, triangle-masked on the first 128),
  row-sharded output projection producing a partial [DIM, S] f32 output.
Host sums the 4 partials per batch (the row-parallel all-reduce) + transposes.
"""

import numpy as np
import ml_dtypes
from contextlib import ExitStack

import concourse.bass as bass
import concourse.tile as tile
from concourse import bacc, mybir, bass_utils, masks

B, S, DIM = 2, 2048, 2048
NH, NKV, HD = 16, 4, 128
TPR = 4            # tensor-parallel ranks per batch
LQH = NH // TPR    # 4 local q heads
QB = 512           # q block (free dim of matmuls)
NQB = S // QB      # 4
NDT = DIM // 128   # 16 contraction tiles for the projections
NKT = S // 128     # 16 key tiles
SCALE = 1.0 / float(np.sqrt(HD))

BF = mybir.dt.bfloat16
F32 = mybir.dt.float32


def _build(reps=1, bench_outs=None):
    nc = bacc.Bacc("TRN2", target_bir_lowering=False, debug=False, num_devices=8)

    xt_d = nc.dram_tensor("xt", [DIM, S], BF, kind="ExternalInput").ap()
    wq_d = nc.dram_tensor("wq", [DIM, LQH * HD], BF, kind="ExternalInput").ap()
    wk_d = nc.dram_tensor("wk", [DIM, HD], BF, kind="ExternalInput").ap()
    wv_d = nc.dram_tensor("wv", [DIM, HD], BF, kind="ExternalInput").ap()
    wo_d = nc.dram_tensor("wo", [LQH * HD, DIM], BF, kind="ExternalInput").ap()
    ce_d = nc.dram_tensor("ce", [HD, S], BF, kind="ExternalInput").ap()
    se_d = nc.dram_tensor("se", [HD, S], BF, kind="ExternalInput").ap()
    n_outs = bench_outs if bench_outs is not None else reps
    assert reps <= n_outs
    out_ds = [
        nc.dram_tensor("out" if r == 0 else f"out{r}", [DIM, S], BF,
                       kind="ExternalOutput").ap()
        for r in range(n_outs)
    ]

    with tile.TileContext(nc, trace_sim=False) as tc, ExitStack() as ctx:
        persist = ctx.enter_context(tc.tile_pool(name="persist", bufs=1))
        xt_pool = ctx.enter_context(tc.tile_pool(name="xtq", bufs=3))
        work = ctx.enter_context(tc.tile_pool(name="work", bufs=3))
        expp = ctx.enter_context(tc.tile_pool(name="expp", bufs=12))
        esump = ctx.enter_context(tc.tile_pool(name="esump", bufs=2))
        outp = ctx.enter_context(tc.tile_pool(name="outp", bufs=4))
        # PSUM budget (8 banks total): pacc 2 + pscr 4 + pden 2
        psum = ctx.enter_context(tc.tile_pool(name="psum", bufs=1, space="PSUM"))

        for _rep in range(reps):
          out_d = out_ds[_rep]
          x_src = xt_d if _rep == 0 else out_ds[_rep - 1]
          # ---- persistent SBUF tensors ----
          wq_sb = persist.tile([128, NDT * LQH * HD], BF, tag="wq")   # dt-major blocks of 512
          wk_sb = persist.tile([128, NDT * HD], BF, tag="wk")
          wv_sb = persist.tile([128, NDT * HD], BF, tag="wv")
          wo_sb = persist.tile([128, LQH * DIM], BF, tag="wo")        # h-major blocks of 2048
          ce_sb = persist.tile([128, S], BF, tag="ce")
          se_sb = persist.tile([128, S], BF, tag="se")
          ones_sb = persist.tile([128, 128], BF, tag="ones")

          # Split per (head / q-block generation) so interleaved writers and
          # readers touch DIFFERENT tiles: the tile dependency tracker is
          # tile-granular, and disjoint ranges of one big tile would create
          # false cross-stage dependencies that stall the PE.
          gq0 = _rep * NQB  # global q-block index of this rep's qb=0
          qt_t = {}  # (h, parity) -> [128, QB] roped Q^T
          ot_t = {}  # (h, parity) -> [128, QB] normalized attn out^T
          for h in range(LQH):
              for p in range(2):
                  qt_t[(h, p)] = persist.tile(
                      [128, QB], BF, tag=f"qt{h}_{p}", name="qt")
                  ot_t[(h, p)] = persist.tile(
                      [128, QB], BF, tag=f"ot{h}_{p}", name="ot")
          kt_t = {}  # gq%8 -> [128, QB] roped K^T (4 key tiles each)
          vn_t = {}  # gq%8 -> [128, QB] V natural (4 key tiles each)
          for p in range(8):
              kt_t[p] = persist.tile([128, QB], BF, tag=f"kt{p}", name="kt")
              vn_t[p] = persist.tile([128, QB], BF, tag=f"vn{p}", name="vn")
          vt_t = {}  # gq%2 -> [128, QB] V^T
          for p in range(2):
              vt_t[p] = persist.tile([128, QB], BF, tag=f"vt{p}", name="vt")

          def dma(out_ap, in_ap):
              nc.sync.dma_start(out_ap, in_ap)

          def dma_rows(sb_ap, dram_ap, groups, cols):
              # one DMA for `groups` consecutive 128-row blocks of a row-major
              # DRAM matrix into column-blocks of a [128, groups*cols] SBUF tile
              dma(
                  sb_ap.rearrange("p (t q) -> p t q", t=groups),
                  dram_ap.rearrange("(t p) q -> p t q", p=128),
              )

          xq0 = xt_pool.tile([128, NDT * QB], BF, tag="xq")
          dma_rows(xq0[:, 0:QB], x_src[0:128, 0:QB], 1, QB)
          dma_rows(wq_sb[:, 0:512], wq_d[0:128, :], 1, 512)
          dma_rows(xq0[:, QB:4 * QB], x_src[128:512, 0:QB], 3, QB)
          dma_rows(wq_sb[:, 512:4 * 512], wq_d[128:512, :], 3, 512)
          for g in range(1, 4):
              dma_rows(
                  xq0[:, g * 4 * QB:(g + 1) * 4 * QB],
                  x_src[g * 512:(g + 1) * 512, 0:QB], 4, QB,
              )
              dma_rows(
                  wq_sb[:, g * 4 * 512:(g + 1) * 4 * 512],
                  wq_d[g * 512:(g + 1) * 512, :], 4, 512,
              )
          dma_rows(wk_sb[:], wk_d[:], NDT, HD)
          dma_rows(wv_sb[:], wv_d[:], NDT, HD)
          dma(ce_sb[:], ce_d[:])
          dma(se_sb[:], se_d[:])
          nc.gpsimd.memset(ones_sb[:], 1.0)

          def rope(psum_in, out_ap, qb):
              """out = in * [c;c] + halfswap(in) * [-s;s], written as bf16."""
              pre = work.tile([128, QB], BF, tag="pre")
              nc.scalar.copy(pre[:], psum_in[:])
              swp = work.tile([128, QB], BF, tag="swp")
              dma(swp[0:64, :], pre[64:128, :])
              dma(swp[64:128, :], pre[0:64, :])
              t1 = work.tile([128, QB], F32, tag="t1")
              nc.vector.tensor_mul(t1[:], pre[:], ce_sb[:, qb * QB:(qb + 1) * QB])
              t2 = work.tile([128, QB], F32, tag="t2")
              nc.vector.tensor_mul(t2[:], swp[:], se_sb[:, qb * QB:(qb + 1) * QB])
              nc.vector.tensor_add(out_ap, t1[:], t2[:])

          # ---- projection pieces (emitted as chunks inside the fused loop) --
          def proj_Q(qb, xq, h):
              pq = psum.tile([128, QB], F32, tag="pacc", bufs=4)
              for dt in range(NDT):
                  nc.tensor.matmul(
                      pq[:],
                      wq_sb[:, dt * 512 + h * 128: dt * 512 + (h + 1) * 128],
                      xq[:, dt * QB:(dt + 1) * QB],
                      start=(dt == 0),
                      stop=(dt == NDT - 1),
                  )
              rope(pq, qt_t[(h, (gq0 + qb) % 2)][:, :], qb)

          def proj_K(qb, xq):
              pk = psum.tile([128, QB], F32, tag="pacc", bufs=4)
              for dt in range(NDT):
                  nc.tensor.matmul(
                      pk[:],
                      wk_sb[:, dt * 128:(dt + 1) * 128],
                      xq[:, dt * QB:(dt + 1) * QB],
                      start=(dt == 0),
                      stop=(dt == NDT - 1),
                  )
              rope(pk, kt_t[(gq0 + qb) % 8][:, :], qb)

          def proj_V(qb, xq):
              pv = psum.tile([128, QB], F32, tag="pacc", bufs=4)
              for dt in range(NDT):
                  nc.tensor.matmul(
                      pv[:],
                      wv_sb[:, dt * 128:(dt + 1) * 128],
                      xq[:, dt * QB:(dt + 1) * QB],
                      start=(dt == 0),
                      stop=(dt == NDT - 1),
                  )
              vt = vt_t[(gq0 + qb) % 2]
              vn = vn_t[(gq0 + qb) % 8]
              nc.scalar.copy(vt[:, :], pv[:])
              for j in range(4):  # V natural via DMA XBAR
                  nc.sync.dma_start(
                      vn[:, j * 128:(j + 1) * 128],
                      vt[:, j * 128:(j + 1) * 128],
                      transpose=True,
                  )

          def proj_stage(qb, xq):
              # pipeline prologue (qb=0 only): K and V first so their rope /
              # transpose chains finish during the Q projections
              proj_K(qb, xq)
              proj_V(qb, xq)
              for h in range(LQH):
                  proj_Q(qb, xq, h)

          # deferred denominator work: (esum, po, ot slice) per finished head.
          # Flushed a safe distance later so the PE never waits on DVE.
          pending_den = []

          def flush_den():
              while pending_den:
                  esum_t, po_t, oslice = pending_den.pop(0)
                  pden = psum.tile([128, QB], F32, tag="pden", bufs=1)
                  nc.tensor.matmul(
                      pden[:], ones_sb[:], esum_t[:], start=True, stop=True,
                  )
                  rec = work.tile([128, QB], F32, tag="rec")
                  nc.vector.reciprocal(rec[:], pden[:])
                  nc.vector.tensor_mul(oslice, po_t[:], rec[:])

          _p3tags = [("pacc", 4), ("pscr", 3), ("pden", 1), ("pscr", 3)]

          def out_grp(qb, grp):
              obg = outp.tile([128, 4 * QB], BF, tag="obg", name="obg")
              for i in range(4):
                  dct = grp * 4 + i
                  _t, _b = _p3tags[dct % 4]
                  pw = psum.tile([128, QB], F32, tag=_t, bufs=_b, name="pw")
                  for h in range(LQH):
                      nc.tensor.matmul(
                          pw[:],
                          wo_sb[:, h * DIM + dct * 128: h * DIM + (dct + 1) * 128],
                          ot_sb[:, h * S + qb * QB: h * S + (qb + 1) * QB],
                          start=(h == 0),
                          stop=(h == LQH - 1),
                      )
                  nc.vector.tensor_copy(obg[:, i * QB:(i + 1) * QB], pw[:])
              dma(
                  out_d[grp * 512:(grp + 1) * 512,
                        qb * QB:(qb + 1) * QB].rearrange(
                      "(t p) q -> p t q", p=128),
                  obg.rearrange("p (t q) -> p t q", t=4),
              )

          def fused_stage(qb, xq_next):
              """Attention(qb) head-streams interleaved, at head boundaries,
              with out-projection(qb-1) groups and projection(qb+1) chunks —
              keeps the PE fed while the exp stream (ACT-bound) drains.

              Work items: (kt, q_off, q_w, masked); fulls first, then the
              128-granular causal diagonal: item j covers q in [128j, 512)
              against key tile 4qb+j, triangle-masked on its first 128 cols.
              """
              items = [(kt, 0, QB, False) for kt in range(4 * qb)]
              items += [
                  (4 * qb + j, j * 128, QB - j * 128, True) for j in range(4)
              ]
              last = len(items) - 1
              SKEW = 5  # PV matmuls trail the score/exp stream

              state = {}  # h -> (po, esum)
              ring = {}

              def pv_mm(h, idx, es):
                  kt, q_off, q_w, _ = items[idx]
                  po_t, esum_t = state[h]
                  vn = vn_t[(gq0 + kt // 4) % 8]
                  nc.tensor.matmul(
                      po_t[:, q_off:q_off + q_w],
                      vn[:, (kt % 4) * 128:(kt % 4 + 1) * 128],
                      es[:, :q_w],
                      start=(idx == 0),
                      stop=(idx == last),
                  )
                  if idx == last:
                      pending_den.append((
                          esum_t, po_t,
                          ot_t[(h, (gq0 + qb) % 2)][:, :],
                      ))

              def emit_item(g, h, idx):
                  kt, q_off, q_w, masked = items[idx]
                  ktile = kt_t[(gq0 + kt // 4) % 8]
                  pscr = psum.tile([128, QB], F32, tag="pscr", bufs=3,
                                   name="pscr")
                  nc.tensor.matmul(
                      pscr[:, :q_w],
                      ktile[:, (kt % 4) * 128:(kt % 4 + 1) * 128],
                      qt_t[(h, (gq0 + qb) % 2)][:, q_off:q_off + q_w],
                      start=True,
                      stop=True,
                  )
                  es = expp.tile([128, QB], BF, tag="es", name="es")
                  nc.scalar.activation(
                      es[:, :q_w], pscr[:, :q_w],
                      mybir.ActivationFunctionType.Exp, scale=SCALE,
                  )
                  if masked:  # causal triangle on the first 128 columns
                      nc.gpsimd.affine_select(
                          out=es[:, :128],
                          in_=es[:, :128],
                          compare_op=mybir.AluOpType.is_ge,
                          fill=0.0,
                          base=0,
                          channel_multiplier=-1,
                          pattern=[[1, 128]],
                      )
                  esum_t = state[h][1]
                  if idx == 0:
                      nc.vector.tensor_copy(esum_t[:, :q_w], es[:, :q_w])
                  else:
                      nc.vector.tensor_add(
                          esum_t[:, q_off:q_off + q_w],
                          esum_t[:, q_off:q_off + q_w],
                          es[:, :q_w],
                      )
                  ring[g] = (h, idx, es)
                  if g >= SKEW:
                      pv_mm(*ring.pop(g - SKEW))

              n = len(items)
              for h in range(LQH):
                  state[h] = (
                      psum.tile([128, QB], F32, tag="pacc", bufs=4, name="po"),
                      esump.tile([128, QB], BF, tag="esum", name="esum"),
                  )
                  for idx in range(n):
                      emit_item(h * n + idx, h, idx)
                  # ---- chunk boundary: fill ACT-bound gaps with other work
                  flush_den()
                  if qb > 0:
                      out_grp(qb - 1, h)
                  if h < LQH - 1:
                      if xq_next is not None:
                          if h == 0:
                              proj_K(qb + 1, xq_next)
                          elif h == 1:
                              proj_V(qb + 1, xq_next)
                          else:
                              proj_Q(qb + 1, xq_next, 0)
                              proj_Q(qb + 1, xq_next, 1)
                  else:
                      # drain the PV ring (exp tail completed during out_grp)
                      for g in sorted(ring):
                          pv_mm(*ring.pop(g))
                      if xq_next is not None:
                          proj_Q(qb + 1, xq_next, 2)
                          proj_Q(qb + 1, xq_next, 3)

          xqs = {0: xq0}
          for qb in range(NQB):
              if qb == 0:
                  proj_stage(0, xqs.pop(0))
              if qb + 1 < NQB:  # prefetch next x block
                  xq = xt_pool.tile([128, NDT * QB], BF, tag="xq")
                  for g in range(4):
                      dma_rows(
                          xq[:, g * 4 * QB:(g + 1) * 4 * QB],
                          x_src[g * 512:(g + 1) * 512, (qb + 1) * QB:(qb + 2) * QB],
                          4, QB,
                      )
                  xqs[qb + 1] = xq
              if qb == 0:  # wo lands during attention(0)
                  for h in range(LQH):
                      dma(
                          wo_sb[:, h * DIM:(h + 1) * DIM],
                          wo_d[h * 128:(h + 1) * 128, :],
                      )
              fused_stage(qb, xqs.get(qb + 1))
          flush_den()
          for grp in range(4):
              out_grp(NQB - 1, grp)

    nc.compile()
    return nc


_NC_CACHE = []


def _get_nc():
    if not _NC_CACHE:
        _NC_CACHE.append(_build())
    return _NC_CACHE[0]


# head_dim permutation: evens then odds; applied to wq/wk columns so RoPE's
# pair rotation becomes a partition half-swap (scores are invariant to a
# shared permutation of the contraction dim)
_PERM = np.concatenate([np.arange(0, HD, 2), np.arange(1, HD, 2)])


def _make_in_maps(x, cos, sin, wq, wk, wv, wo):
    bf = ml_dtypes.bfloat16
    ct = np.ascontiguousarray(cos.T)  # [64, S]
    st = np.ascontiguousarray(sin.T)
    ce = np.concatenate([ct, ct], axis=0).astype(bf)        # [128, S]
    se = np.concatenate([-st, st], axis=0).astype(bf)
    xt = [np.ascontiguousarray(x[b].T).astype(bf) for b in range(B)]
    wqp = wq.reshape(DIM, NH, HD)[:, :, _PERM].reshape(DIM, NH * HD)
    wkp = wk.reshape(DIM, NKV, HD)[:, :, _PERM].reshape(DIM, NKV * HD)
    in_maps = []
    for c in range(8):
        b, r = divmod(c, TPR)
        in_maps.append(
            {
                "xt": xt[b],
                "wq": np.ascontiguousarray(wqp[:, r * 512:(r + 1) * 512]).astype(bf),
                "wk": np.ascontiguousarray(wkp[:, r * 128:(r + 1) * 128]).astype(bf),
                "wv": np.ascontiguousarray(wv[:, r * 128:(r + 1) * 128]).astype(bf),
                "wo": np.ascontiguousarray(wo[r * 512:(r + 1) * 512, :]).astype(bf),
                "ce": ce,
                "se": se,
            }
        )
    return in_maps


def _assemble(results):
    full = np.empty((B, S, DIM), np.float32)
    for b in range(B):
        acc = results[TPR * b]["out"].astype(np.float32)
        for r in range(1, TPR):
            acc += results[TPR * b + r]["out"].astype(np.float32)
        full[b] = acc.T
    return full


def kernel(x, cos, sin, wq, wk, wv, wo):
    x = np.asarray(x, np.float32)
    cos = np.asarray(cos, np.float32)
    sin = np.asarray(sin, np.float32)
    wq = np.asarray(wq, np.float32)
    wk = np.asarray(wk, np.float32)
    wv = np.asarray(wv, np.float32)
    wo = np.asarray(wo, np.float32)

    nc = _get_nc()
    in_maps = _make_in_maps(x, cos, sin, wq, wk, wv, wo)
    res = bass_utils.run_bass_kernel_spmd(nc, in_maps, core_ids=list(range(8)))
    return _assemble(res.results)


def run_traced(inputs):
    """Timing/profiling helper for test.py (not used by the grader)."""
    nc = _get_nc()
    in_maps = _make_in_maps(
        np.asarray(inputs["x"], np.float32),
        np.asarray(inputs["cos"], np.float32),
        np.asarray(inputs["sin"], np.float32),
        np.asarray(inputs["wq"], np.float32),
        np.asarray(inputs["wk"], np.float32),
        np.asarray(inputs["wv"], np.float32),
        np.asarray(inputs["wo"], np.float32),
    )
    res = bass_utils.run_bass_kernel_spmd(
        nc, in_maps, core_ids=list(range(8)), trace=True
    )
    return res
